# revision 15
# baseline (speedup 1.0000x reference)
"""Trainium2 Bass kernel for nn_Decoder_80814104642079 (GNN message passing).

Computes out = spmm(A, spmm(A, feat @ W1) @ W2) for a sparse adjacency A with
3.2M edges over 100K nodes, via the identity out = (A @ (A @ feat)) @ (W1@W2),
so both sparse products run at feature width 32.

Distribution over 8 NeuronCores (single SPMD NEFF):
- destinations sharded: core k owns rows [12500k, 12500(k+1))
- per core, edges grouped into 4 source-chunk passes (int16 gather indices),
  sorted by destination block; chunks of 128 edges
- gather: 4-queue SWDGE dma_gather from a 256B/row bf16 hi/lo table
- scatter: per chunk, DVE builds onehot[e,dst]*val (tensor_scalar on an iota
  tile); PE accumulates onehot^T @ gathered(hi) + onehot^T @ gathered(lo)
  into a whole-shard PSUM accumulator [128, 98*32] f32
- t = A@feat exchanged via device AllGather (bf16 hi/lo)
- tail: u^T per block via PE transpose, out^T = Wc^T-matmul, Wc = W1@W2
- chunk budgets per (pass, dst-block) are maxed over cores so one program
  serves all cores; per-core differences live in the input streams only
"""

import sys

import numpy as np

sys.path.insert(0, "/opt/trn_rl_repo")

import concourse.bacc as bacc
import concourse.bass as bass
import concourse.mybir as mybir
from concourse import library_config
from contextlib import ExitStack

BLK = 128          # dst block size / matmul K
GB = 32            # chunks (columns) per dma_gather batch
NSTAGE = 4         # gather staging ring slots
NOH = 64           # one-hot ring (chunks); NOH/GB batches resident
NPASS = 4          # source-chunk passes (int16 index range)
ELEM = 128         # bf16 units per table row (256B)
NQUEUES = 4
N_CORES = 8


def _wrap_idx(seg):
    n = seg.shape[0]
    w16 = seg.reshape(n // 16, 16).T
    return np.tile(w16, (8, 1)).astype(np.int16)


def build_schedule(rows, cols, vals, n_cores, shard, sh_blocks, tbl_rows,
                   col_to_table):
    """Uniform-budget chunk schedule for one spmm. Returns per-core streams."""
    import ml_dtypes

    E = rows.shape[0]
    chunk_rows = tbl_rows // NPASS
    assert chunk_rows <= 32767
    core_of = np.minimum(rows // shard, n_cores - 1)
    row_l = rows - core_of * shard
    tcol = col_to_table(cols)
    pass_of = tcol // chunk_rows
    idx_in = (tcol - pass_of * chunk_rows).astype(np.int16)
    blk_of = row_l // BLK
    nblk = sh_blocks

    key = (core_of.astype(np.int64) * NPASS + pass_of) * nblk + blk_of
    counts = np.bincount(key, minlength=n_cores * NPASS * nblk).reshape(
        n_cores, NPASS, nblk)
    C = (-(-counts // BLK)).max(axis=0)          # [NPASS, nblk]
    totch_pass = C.sum(axis=1)
    totch_pass_pad = -(-totch_pass // GB) * GB   # whole batches per pass
    TOTCH = int(totch_pass_pad.sum())
    NBATCH = TOTCH // GB
    nb_pass = (totch_pass_pad // GB).astype(np.int64)

    chunk_blocks = np.zeros(TOTCH, dtype=np.int32)
    pass_starts = np.zeros(NPASS + 1, dtype=np.int64)
    off = 0
    for P in range(NPASS):
        pass_starts[P] = off
        cb = np.repeat(np.arange(nblk), C[P])
        chunk_blocks[off:off + cb.shape[0]] = cb
        off += int(totch_pass_pad[P])
    pass_starts[NPASS] = off

    blk_chunk_start = np.zeros((NPASS, nblk), dtype=np.int64)
    for P in range(NPASS):
        blk_chunk_start[P] = pass_starts[P] + np.concatenate(
            ([0], np.cumsum(C[P])[:-1]))

    order = np.lexsort((row_l, blk_of, pass_of, core_of))
    so_core = core_of[order]
    so_pass = pass_of[order]
    so_blk = blk_of[order]
    so_rowm = (row_l[order] % BLK).astype(np.float32)
    so_idx = idx_in[order]
    so_val = vals[order].astype(np.float32)
    vhi_b = so_val.astype(ml_dtypes.bfloat16)
    vres = so_val - vhi_b.astype(np.float32)

    sort_key = (so_core.astype(np.int64) * NPASS + so_pass) * nblk + so_blk
    grp_start = np.searchsorted(sort_key, np.arange(n_cores * NPASS * nblk),
                                side="left")
    local_rank = np.arange(E, dtype=np.int64) - grp_start[sort_key]
    g_chunk = blk_chunk_start[so_pass, so_blk] + local_rank // BLK
    g_part = local_rank % BLK

    idx_all = np.zeros((n_cores, TOTCH * BLK), dtype=np.int16)
    meta_all = np.zeros((n_cores, 128, TOTCH, 4), dtype=np.float32)
    flatpos = g_chunk * BLK + g_part
    for k in range(n_cores):
        m = so_core == k
        idx_all[k, flatpos[m]] = so_idx[m]
        meta_all[k, g_part[m], g_chunk[m], 0] = so_rowm[m]
        meta_all[k, g_part[m], g_chunk[m], 1] = so_val[m]
        meta_all[k, g_part[m], g_chunk[m], 2] = vres[m]

    idx_wrapped = np.zeros((n_cores, NBATCH, 128, GB * 8), dtype=np.int16)
    for k in range(n_cores):
        for bi in range(NBATCH):
            idx_wrapped[k, bi] = _wrap_idx(
                idx_all[k, bi * GB * BLK:(bi + 1) * GB * BLK])

    return dict(idx_wrapped=idx_wrapped, meta=meta_all,
                chunk_blocks=chunk_blocks, nb_pass=nb_pass,
                pass_starts=pass_starts, TOTCH=TOTCH, NBATCH=NBATCH)


def build_nc(sch1, sch2, n_cores, sh_blocks, tbl1_rows, tbl2_rows, precise=1):
    MW = 2 if precise == 1 else 3
    SHARD_P = sh_blocks * BLK
    nc = bacc.Bacc("TRN2", num_swdge_queues=NQUEUES)
    f32, bf16, i16 = mybir.dt.float32, mybir.dt.bfloat16, mybir.dt.int16

    NB1, NB2 = sch1["NBATCH"], sch2["NBATCH"]
    TC1, TC2 = sch1["TOTCH"], sch2["TOTCH"]
    NBP = int(max(sch1["nb_pass"].max(), sch2["nb_pass"].max()))
    TCP = NBP * GB

    feat_tbl = nc.dram_tensor("feat_tbl", [tbl1_rows, ELEM], bf16,
                              kind="ExternalInput")
    w1t = nc.dram_tensor("w1t", [64, 32], f32, kind="ExternalInput")
    w2 = nc.dram_tensor("w2", [64, 128], f32, kind="ExternalInput")
    idx1 = nc.dram_tensor("idx1", [NB1, 128, GB * 8], i16, kind="ExternalInput")
    meta1 = nc.dram_tensor("meta1", [128, TC1, MW], f32, kind="ExternalInput")
    idx2 = nc.dram_tensor("idx2", [NB2, 128, GB * 8], i16, kind="ExternalInput")
    meta2 = nc.dram_tensor("meta2", [128, TC2, MW], f32, kind="ExternalInput")
    out_ext = nc.dram_tensor("outT", [128, SHARD_P], f32, kind="ExternalOutput")

    t_loc = nc.dram_tensor("t_loc", [SHARD_P, ELEM], bf16)
    t_all = nc.dram_tensor("t_all", [tbl2_rows, ELEM], bf16)
    assert tbl2_rows == SHARD_P * n_cores

    with ExitStack() as ctx:
        sb = lambda name, shape, dt: ctx.enter_context(
            nc.sbuf_tensor(name, shape, dt))
        sem = lambda name: ctx.enter_context(nc.semaphore(name))

        iota_bf = sb("iota_bf", [128, BLK], bf16)
        iota_i = sb("iota_i", [128, BLK], i16)
        g_st = sb("g_st", [128, NSTAGE, GB, ELEM], bf16)
        oh_ring = sb("oh_ring", [128, NOH, BLK], bf16)
        oh2_ring = sb("oh2_ring", [128, NOH, BLK], bf16) if precise >= 2 else None
        idx_sb = sb("idx_sb", [128, 2, NBP, GB * 8], i16)
        meta_sb = sb("meta_sb", [128, 2, TCP, MW], f32)
        tst = sb("tst", [128, sh_blocks, 64], bf16)
        u_sb = sb("u_sb", [128, sh_blocks, 32], f32)
        ident = sb("ident", [128, 128], f32)
        w1t_sb = sb("w1t_sb", [64, 32], f32)
        w2_sb = sb("w2_sb", [64, 128], f32)
        wc_sb = sb("wc_sb", [32, 128], f32)
        uT_sb = sb("uT_sb", [32, SHARD_P], f32)
        o_sb = sb("o_sb", [128, 2, 512], f32)

        psum_acc = ctx.enter_context(
            nc.psum_tensor("psum_acc", [128, sh_blocks * 32], f32))
        psum_sm = ctx.enter_context(nc.psum_tensor("psum_sm", [128, 512], f32))

        s_w = sem("s_w")
        s_ld = [sem("s_ld0"), sem("s_ld1")]
        s_tw = sem("s_tw")
        s_out = sem("s_out")
        s_gat = [sem(f"s_gat{q}") for q in range(NQUEUES)]
        s_dve = sem("s_dve")
        s_pe = sem("s_pe")
        s_mm = sem("s_mm")
        s_evac = sem("s_evac")
        s_cc = sem("s_cc")
        s_zero = sem("s_zero")

        ld_cnt = [0, 0]
        out_cnt = [0]

        def load_slot(sl, dst_ap, src_ap):
            nc.sync.dma_start(out=dst_ap, in_=src_ap).then_inc(s_ld[sl], 16)
            ld_cnt[sl] += 16
            return ld_cnt[sl]

        s_init = sem("s_init")
        nc.gpsimd.load_library(library_config.mlp)
        nc.gpsimd.iota(iota_i[:], pattern=[[1, BLK]], base=0,
                       channel_multiplier=0)
        nc.gpsimd.memset(ident[:], 0.0)
        nc.gpsimd.drain()
        nc.gpsimd.affine_select(
            out=ident[:], in_=ident[:],
            compare_op=mybir.AluOpType.not_equal, fill=1.0, base=0,
            pattern=[[-1, 128]], channel_multiplier=1)
        nc.gpsimd.drain()
        nc.gpsimd.sem_inc(s_init, 1)
        nc.vector.wait_ge(s_init, 1)
        nc.vector.tensor_copy(out=iota_bf[:], in_=iota_i[:])
        nc.vector.drain()

        nc.sync.dma_start(out=w1t_sb[:], in_=w1t[:]).then_inc(s_w, 16)
        nc.sync.dma_start(out=w2_sb[:], in_=w2[:]).then_inc(s_w, 16)

        nc.tensor.wait_ge(s_w, 32)
        nc.tensor.matmul(out=psum_sm[:32, :128], lhsT=w1t_sb[:], rhs=w2_sb[:],
                         start=True, stop=True).then_inc(s_mm, 1)  # mm=1
        nc.scalar.wait_ge(s_mm, 1)
        nc.scalar.activation(out=wc_sb[:], in_=psum_sm[:32, :128],
                             func=mybir.ActivationFunctionType.Copy
                             ).then_inc(s_evac, 1)       # evac=1

        st = dict(pe=0, dve=0, g=0, gq=[0] * NQUEUES,
                  zero=0, evac=1, cc=0, tw=0, out=0, mm=1)

        def run_spmm(sch, idx_dram, meta_dram, table, tbl_rows, zt):
            chunk_blocks = sch["chunk_blocks"]
            nb_pass = sch["nb_pass"]
            pass_starts = sch["pass_starts"]
            chunk_rows = tbl_rows // NPASS
            base_pe = st["pe"]

            # load pass 0 streams
            io_p = [0, 0]
            load_slot(0, idx_sb[:, 0, :int(nb_pass[0]), :],
                      idx_dram[0:int(nb_pass[0])].transpose([1, 0, 2]))
            io_p[0] = load_slot(
                0, meta_sb[:, 0, :int(nb_pass[0]) * GB, :],
                meta_dram[:, int(pass_starts[0]):int(pass_starts[1]), :])

            gbi = 0  # batch index within spmm
            for P in range(NPASS):
                nbp = int(nb_pass[P])
                sl = P % 2
                # prefetch next pass streams into other slot
                if P + 1 < NPASS:
                    nxt = int(nb_pass[P + 1])
                    if P >= 1:
                        # WAR: slot (P+1)%2 last used by pass P-1 -> wait PE
                        nc.sync.wait_ge(s_pe, base_pe + gbi)
                    if nxt > 0:
                        load_slot((P + 1) % 2,
                                  idx_sb[:, (P + 1) % 2, :nxt, :],
                                  idx_dram[gbi + nbp:gbi + nbp + nxt].transpose(
                                      [1, 0, 2]))
                        io_p[(P + 1) % 2] = load_slot(
                            (P + 1) % 2,
                            meta_sb[:, (P + 1) % 2, :nxt * GB, :],
                            meta_dram[:, int(pass_starts[P + 1]):
                                      int(pass_starts[P + 1]) + nxt * GB, :])
                if nbp == 0:
                    continue
                nc.gpsimd.wait_ge(s_ld[sl], io_p[sl])
                nc.vector.wait_ge(s_ld[sl], io_p[sl])
                tbl_ap = table[P * chunk_rows:(P + 1) * chunk_rows, :]

                for bp in range(nbp):
                    bi = gbi + bp
                    gi = st["g"] + bi
                    slot = gi % NSTAGE
                    q = gi % NQUEUES
                    if bi >= NSTAGE:
                        nc.gpsimd.wait_ge(s_pe, base_pe + bi - NSTAGE + 1)
                    nc.gpsimd.dma_gather(
                        g_st[:, slot, :, :], tbl_ap,
                        idx_sb[:, sl, bp, :], GB * BLK, GB * BLK, ELEM,
                        elem_step=ELEM, single_packet=False, queue_num=q,
                    ).then_inc(s_gat[q], 16)
                    st["gq"][q] += 1

                    dbi = st["dve"] + bi
                    ring_b = NOH // GB
                    if bi - ring_b >= 0:
                        nc.vector.wait_ge(s_pe, base_pe + bi - ring_b + 1)
                    if bi == 0:
                        nc.vector.wait_ge(s_zero, zt)
                    last_oh = None
                    for c in range(GB):
                        cpl = bp * GB + c
                        ohslot = (dbi * GB + c) % NOH
                        last_oh = nc.vector.tensor_scalar(
                            out=oh_ring[:, ohslot, :],
                            in0=iota_bf[:],
                            scalar1=meta_sb[:, sl, cpl, 0:1],
                            scalar2=meta_sb[:, sl, cpl, 1:2],
                            op0=mybir.AluOpType.is_equal,
                            op1=mybir.AluOpType.mult)
                        if precise >= 2:
                            last_oh = nc.vector.tensor_scalar(
                                out=oh2_ring[:, ohslot, :],
                                in0=iota_bf[:],
                                scalar1=meta_sb[:, sl, cpl, 0:1],
                                scalar2=meta_sb[:, sl, cpl, 2:3],
                                op0=mybir.AluOpType.is_equal,
                                op1=mybir.AluOpType.mult)
                    last_oh.then_inc(s_dve, 1)

                    nc.tensor.wait_ge(s_dve, st["dve"] + bi + 1)
                    nc.tensor.wait_ge(s_gat[q], 16 * st["gq"][q])
                    if bi == 0:
                        nc.tensor.wait_ge(s_zero, zt)
                    last_mm = [None]
                    for c in range(GB):
                        ch = int(pass_starts[P]) + bp * GB + c
                        b = int(chunk_blocks[ch])
                        ohslot = ((st["dve"] + bi) * GB + c) % NOH
                        ps = psum_acc[:, b * 32:(b + 1) * 32]
                        last_mm[0] = nc.tensor.matmul(
                            out=ps, lhsT=oh_ring[:, ohslot, :],
                            rhs=g_st[:, slot, c, 0:32],
                            start=False, stop=False, skip_group_check=True)
                        last_mm[0] = nc.tensor.matmul(
                            out=ps, lhsT=oh_ring[:, ohslot, :],
                            rhs=g_st[:, slot, c, 32:64],
                            start=False, stop=False, skip_group_check=True)
                        if precise >= 2:
                            last_mm[0] = nc.tensor.matmul(
                                out=ps, lhsT=oh2_ring[:, ohslot, :],
                                rhs=g_st[:, slot, c, 0:32],
                                start=False, stop=False, skip_group_check=True)
                    last_mm[0].then_inc(s_pe, 1)
                gbi += nbp

            st["pe"] += gbi
            st["dve"] += gbi
            st["g"] += gbi

        import os as _os
        REPEAT = int(_os.environ.get("BASS_GNN_REPEAT", "1"))
        for _rep in range(REPEAT):
            # zero psum accumulator for spmm1
            if _rep == 0:
                nc.vector.memset(psum_acc[:], 0).then_inc(s_zero, 1)
            else:
                nc.vector.wait_ge(s_evac, st["evac"])
                nc.vector.memset(psum_acc[:], 0).then_inc(s_zero, 1)
            st["zero"] += 1

            # ================= SPMM 1 =================
            run_spmm(sch1, idx1, meta1, feat_tbl, tbl1_rows, zt=st["zero"])
            pe1 = st["pe"]

            nc.scalar.wait_ge(s_pe, pe1)
            last = None
            for b in range(sh_blocks):
                last = nc.scalar.activation(
                    out=tst[:, b, 0:32], in_=psum_acc[:, b * 32:(b + 1) * 32],
                    func=mybir.ActivationFunctionType.Copy)
            last.then_inc(s_evac, 1)
            st["evac"] += 1
            nc.vector.wait_ge(s_evac, st["evac"])
            nc.vector.wait_ge(s_pe, pe1)
            last = None
            for b in range(sh_blocks):
                last = nc.vector.tensor_tensor(
                    out=tst[:, b, 32:64], in0=psum_acc[:, b * 32:(b + 1) * 32],
                    in1=tst[:, b, 0:32], op=mybir.AluOpType.subtract)
            last.then_inc(s_evac, 1)
            st["evac"] += 1

            nc.sync.wait_ge(s_evac, st["evac"])
            nc.sync.dma_start(
                out=t_loc[:].rearrange("(b p) e -> p b e", p=128)[:, :, 0:64],
                in_=tst[:]).then_inc(s_tw, 16)
            st["tw"] += 16
            nc.gpsimd.wait_ge(s_tw, st["tw"])
            nc.gpsimd.collective_compute(
                "AllGather", mybir.AluOpType.bypass,
                replica_groups=[list(range(n_cores))],
                ins=[t_loc[:]], outs=[t_all[:]],
            ).then_inc(s_cc, 1)
            st["cc"] += 1
            nc.gpsimd.wait_ge(s_cc, st["cc"])

            nc.vector.wait_ge(s_evac, st["evac"])
            nc.vector.memset(psum_acc[:], 0).then_inc(s_zero, 1)
            st["zero"] += 1

            # ================= SPMM 2 =================
            run_spmm(sch2, idx2, meta2, t_all, tbl2_rows, zt=st["zero"])
            pe2 = st["pe"]

            nc.scalar.wait_ge(s_pe, pe2)
            last = None
            for b in range(sh_blocks):
                last = nc.scalar.activation(
                    out=u_sb[:, b, :], in_=psum_acc[:, b * 32:(b + 1) * 32],
                    func=mybir.ActivationFunctionType.Copy)
            last.then_inc(s_evac, 1)
            st["evac"] += 1

            # transposes
            nc.tensor.wait_ge(s_init, 1)
            nc.tensor.wait_ge(s_evac, st["evac"])
            for b in range(sh_blocks):
                nc.tensor.transpose(out=psum_sm[:32, 0:128], in_=u_sb[:, b, :],
                                    identity=ident[:]).then_inc(s_mm, 1)
                st["mm"] += 1
                nc.scalar.wait_ge(s_mm, st["mm"])
                nc.scalar.activation(out=uT_sb[:, b * 128:(b + 1) * 128],
                                     in_=psum_sm[:32, 0:128],
                                     func=mybir.ActivationFunctionType.Copy
                                     ).then_inc(s_evac, 1)
                st["evac"] += 1
                if b + 1 < sh_blocks:
                    nc.tensor.wait_ge(s_evac, st["evac"])

            # outT = Wc^T @ uT, chunks of 512 columns
            ncol = SHARD_P
            nchunks = -(-ncol // 512)
            io_marks = []
            for j in range(nchunks):
                c0, c1 = j * 512, min((j + 1) * 512, ncol)
                w = c1 - c0
                ping = j % 2
                nc.tensor.wait_ge(s_evac, st["evac"])
                nc.tensor.matmul(out=psum_sm[:, 0:w], lhsT=wc_sb[:],
                                 rhs=uT_sb[:, c0:c1], start=True, stop=True,
                                 skip_group_check=True).then_inc(s_mm, 1)
                st["mm"] += 1
                nc.scalar.wait_ge(s_mm, st["mm"])
                if j >= 2:
                    nc.scalar.wait_ge(s_out, io_marks[j - 2])
                nc.scalar.activation(out=o_sb[:, ping, :w],
                                     in_=psum_sm[:, 0:w],
                                     func=mybir.ActivationFunctionType.Copy
                                     ).then_inc(s_evac, 1)
                st["evac"] += 1
                nc.sync.wait_ge(s_evac, st["evac"])
                nc.sync.dma_start(out=out_ext[:, c0:c1],
                                  in_=o_sb[:, ping, :w]).then_inc(s_out, 16)
                out_cnt[0] += 16
                io_marks.append(out_cnt[0])
        nc.sync.wait_ge(s_out, out_cnt[0])
    nc.compile()
    return nc


def _prep_all(feat, W1, W2, edge_row, edge_col, edge_val, n_cores, precise=1):
    MW = 2 if precise == 1 else 3
    import ml_dtypes

    N = feat.shape[0]
    shard = N // n_cores
    sh_blocks = -(-shard // BLK)
    SHARD_P = sh_blocks * BLK

    tbl1_rows = -(-N // (NPASS * 16)) * (NPASS * 16)
    assert tbl1_rows // NPASS <= 32767
    x32 = feat.astype(np.float32)
    hi = x32.astype(ml_dtypes.bfloat16)
    lo = (x32 - hi.astype(np.float32)).astype(ml_dtypes.bfloat16)
    feat_tbl = np.zeros((tbl1_rows, ELEM), dtype=ml_dtypes.bfloat16)
    feat_tbl[:N, 0:32] = hi
    feat_tbl[:N, 32:64] = lo

    tbl2_rows = SHARD_P * n_cores
    assert tbl2_rows % NPASS == 0 and tbl2_rows // NPASS <= 32767

    rows = edge_row.astype(np.int64)
    cols = edge_col.astype(np.int64)
    vals = edge_val.astype(np.float32)

    sch1 = build_schedule(rows, cols, vals, n_cores, shard, sh_blocks,
                          tbl1_rows, col_to_table=lambda c: c)

    def col2(c):
        k = np.minimum(c // shard, n_cores - 1)
        return k * SHARD_P + (c - k * shard)

    sch2 = build_schedule(rows, cols, vals, n_cores, shard, sh_blocks,
                          tbl2_rows, col_to_table=col2)

    in_maps = []
    for k in range(n_cores):
        in_maps.append({
            "feat_tbl": np.asarray(feat_tbl),
            "w1t": np.ascontiguousarray(W1.astype(np.float32).T),
            "w2": np.ascontiguousarray(W2.astype(np.float32)),
            "idx1": sch1["idx_wrapped"][k],
            "meta1": np.ascontiguousarray(sch1["meta"][k][:, :, :MW]),
            "idx2": sch2["idx_wrapped"][k],
            "meta2": np.ascontiguousarray(sch2["meta"][k][:, :, :MW]),
        })
    return sch1, sch2, in_maps, shard, sh_blocks, tbl1_rows, tbl2_rows


def run(feat, W1, W2, edge_row, edge_col, edge_val, n_cores=N_CORES,
        runner=None, precise=2):
    N = feat.shape[0]
    sch1, sch2, in_maps, shard, sh_blocks, t1r, t2r = _prep_all(
        feat, W1, W2, edge_row, edge_col, edge_val, n_cores, precise=precise)
    nc = build_nc(sch1, sch2, n_cores, sh_blocks, t1r, t2r, precise=precise)

    if runner is None:
        from concourse.bass_utils import run_bass_kernel_spmd
        res = run_bass_kernel_spmd(nc, in_maps, core_ids=list(range(n_cores)))
        results = res.results
    else:
        results = runner(nc, in_maps)

    out = np.zeros((N, 128), dtype=np.float32)
    for k in range(n_cores):
        oT = results[k]["outT"]
        out[k * shard:(k + 1) * shard, :] = oT.T[:shard, :]
    return out


def kernel(feat, W1, W2, edge_row, edge_col, edge_val):
    return run(feat, W1, W2, edge_row, edge_col, edge_val)
